# revision 1
# baseline (speedup 1.0000x reference)
"""MoE layer (8 experts, top-2) on 8 Trainium2 NeuronCores, expert-parallel.

Strategy:
  - Host computes the (tiny) gating linear + softmax + top-2 routing,
    mirroring the reference ops so expert selection matches exactly.
  - Tokens are dispatched to their experts on the host (the all-to-all),
    one expert per NeuronCore. Each core runs the 1024->4096->1024 gelu
    FFN for its expert over its routed tokens (padded to a common
    capacity), with all matmuls in float32r at full PE rate.
  - Host applies combine weights and scatter-adds back to token order.

Device layout: activations are kept transposed ([feature, token]) so both
matmuls consume the natural [K, M] weight layout and the phase-1 output
feeds phase-2 without any on-chip transpose. The 4096-wide hidden dim is
processed in quarters: phase 1 produces a quarter of the hidden
activations for ALL tokens (SBUF-resident), phase 2 immediately folds it
into an SBUF-resident partial sum of y. Expert weights therefore stream
from HBM exactly once, which keeps the kernel tensor-engine-bound
(streaming them per token-group was HBM-bound under 8-core contention).
"""

import numpy as np

N_EMBED = 1024
FFN_HIDDEN = 4096
NUM_EXPERTS = 8
TOP_K = 2
P = 128
KT1 = N_EMBED // P       # 8  k-tiles, phase 1
MT1 = FFN_HIDDEN // P    # 32 m-tiles, phase 1
KT2 = FFN_HIDDEN // P    # 32 k-tiles, phase 2
MT2 = N_EMBED // P       # 8  m-tiles, phase 2
QF = 4                   # FFN hidden dim is processed in QF f-quarters so the
                         # resident hT slab stays small and weights stream once

LAST_RESULT = None       # BassKernelResults of the most recent run (debug/profiling)


def _ensure_axon_hooks():
    """Make `antenv.axon_hooks` importable so BASS_TRACE=1 degrades
    gracefully instead of crashing when the image lacks the module."""
    try:
        import antenv.axon_hooks  # noqa: F401
        return
    except ImportError:
        pass
    import sys
    import types

    m = types.ModuleType("antenv.axon_hooks")
    m._hook = None
    m.set_axon_ntff_profile_hook = lambda h: setattr(m, "_hook", h)
    m.get_axon_ntff_profile_hook = lambda: m._hook
    sys.modules["antenv.axon_hooks"] = m
    try:
        from trn_agent_boot.trn_boot import _ntff_profile_via_ctypes

        m.set_axon_ntff_profile_hook(_ntff_profile_via_ctypes("/opt/axon/libaxon_pjrt.so"))
    except Exception:
        pass


def _route(x2d, Wg, bg):
    """Top-2 gating. Mirrors the reference (jax softmax + lax.top_k) so the
    selected experts match it exactly; numpy fallback is numerically
    equivalent up to fp32 rounding."""
    try:
        import jax
        import jax.numpy as jnp

        gate = jax.nn.softmax(jnp.asarray(x2d) @ jnp.asarray(Wg) + jnp.asarray(bg), axis=-1)
        scores, idx = jax.lax.top_k(gate, TOP_K)
        scores = np.asarray(scores, dtype=np.float32)
        idx = np.asarray(idx)
    except Exception:
        logits = x2d @ Wg + bg
        m = logits.max(-1, keepdims=True)
        e = np.exp(logits - m)
        p = e / e.sum(-1, keepdims=True)
        order = np.argsort(-p, axis=-1, kind="stable")
        idx = order[:, :TOP_K]
        scores = np.take_along_axis(p, idx, axis=-1)
    scores = scores / (scores.sum(-1, keepdims=True) + 1e-8)
    return idx.astype(np.int64), scores.astype(np.float32)


def _chunks(width):
    """Split the token capacity (>=256) into matmul free-dim chunks, each
    in [256, 512] so every fp32r matmul runs at full PE rate. Smallest
    chunk first: it gates the very first matmul of the kernel."""
    out, rem = [], width
    while rem > 0:
        if rem <= 512:
            c = rem
        elif rem < 768:
            c = rem - 256
        else:
            c = 512
        out.append(c)
        rem -= c
    out.sort()
    res, off = [], 0
    for c in out:
        res.append((off, c))
        off += c
    return res




def _build_device_program(cap, p2_bf16=False):
    import concourse.tile as tile
    from concourse import bacc, mybir
    from concourse.tile_rust import add_dep_helper

    f32 = mybir.dt.float32
    f32r = mybir.dt.float32r
    p2dt = mybir.dt.bfloat16 if p2_bf16 else f32r
    gelu = mybir.ActivationFunctionType.Gelu_apprx_tanh
    ident = mybir.ActivationFunctionType.Identity

    nc = bacc.Bacc("TRN2", target_bir_lowering=False, debug=False,
                   num_devices=NUM_EXPERTS)

    MQ1 = MT1 // QF  # phase-1 m-tiles (f-tiles) per quarter
    KQ2 = KT2 // QF  # phase-2 k-tiles (f-tiles) per quarter

    xg_d = nc.dram_tensor("xg", [KT1, P, cap], f32r, kind="ExternalInput").ap()
    w1_d = nc.dram_tensor("w1t", [MT1, P, KT1 * P], f32r, kind="ExternalInput").ap()
    # w2 is laid out quarter-sliced: tile (q*MT2 + m) holds the KQ2 k-slices
    # of f-quarter q for output tile m
    w2_d = nc.dram_tensor("w2t", [QF * MT2, P, KQ2 * P], p2dt,
                          kind="ExternalInput").ap()
    b1_d = nc.dram_tensor("b1m", [P, MT1], f32, kind="ExternalInput").ap()
    b2_d = nc.dram_tensor("b2m", [P, MT2], f32, kind="ExternalInput").ap()
    y_d = nc.dram_tensor("yT", [MT2, P, cap], f32, kind="ExternalOutput").ap()

    chunks = _chunks(cap)

    with tile.TileContext(nc) as tc:
        with (
            tc.tile_pool(name="const", bufs=1) as const,
            tc.tile_pool(name="xp", bufs=1) as xp,
            tc.tile_pool(name="hp", bufs=1) as hp,
            tc.tile_pool(name="yp", bufs=1) as yp,
            tc.tile_pool(name="w1p", bufs=4) as w1p,
            tc.tile_pool(name="w2p", bufs=4) as w2p,
            tc.tile_pool(name="psp", bufs=8, space="PSUM") as psp,
            tc.tile_pool(name="op", bufs=6) as op,
        ):
            # one tile per k-tile of x, split at the first chunk boundary;
            # all first-chunk slices are issued before anything else (DMA
            # issue on the sequencer costs ~650ns each) so the first
            # k-accumulation starts as soon as possible
            first_cw = chunks[0][1]
            xg_sbs = []
            for kt in range(KT1):
                xt = xp.tile([P, cap], f32r, name=f"xg{kt}")
                nc.sync.dma_start(xt[:, :first_cw], xg_d[kt, :, :first_cw])
                xg_sbs.append(xt)
            b1_sb = const.tile([P, MT1], f32)
            nc.sync.dma_start(b1_sb[:], b1_d[:, :])
            b2_sb = const.tile([P, MT2], f32)
            nc.sync.dma_start(b2_sb[:], b2_d[:, :])
            for kt in range(KT1):
                nc.sync.dma_start(xg_sbs[kt][:, first_cw:], xg_d[kt, :, first_cw:])

            hT_sb = hp.tile([P, MQ1 * cap], p2dt)
            y_sb = yp.tile([P, MT2 * cap], f32)

            anchor_act = None
            for q in range(QF):
                # phase 1 (quarter q): hT = gelu(W1[:, fq].T @ xT + b1[fq])
                for mq in range(MQ1):
                    m = q * MQ1 + mq
                    w1m = w1p.tile([P, KT1 * P], f32r, tag="w1")
                    nc.gpsimd.dma_start(w1m[:], w1_d[m, :, :])
                    for ci, (cs, cw) in enumerate(chunks):
                        ps = psp.tile([P, cw], f32, tag="ps", name=f"ps{ci}")
                        for kt in range(KT1):
                            nc.tensor.matmul(
                                ps[:],
                                w1m[:, kt * P:(kt + 1) * P],
                                xg_sbs[kt][:, cs:cs + cw],
                                start=(kt == 0),
                                stop=(kt == KT1 - 1),
                            )
                        act = nc.scalar.activation(
                            hT_sb[:, mq * cap + cs:mq * cap + cs + cw],
                            ps[:],
                            gelu,
                            bias=b1_sb[:, m:m + 1],
                        )
                        if q == 0 and mq == 6 and ci == 0:
                            anchor_act = act.ins
                # phase 2 (quarter q): y (+)= W2[fq].T @ hT  [+ b2 on q=0]
                for m in range(MT2):
                    w2m = w2p.tile([P, KQ2 * P], p2dt, tag="w2")
                    w2dma = nc.gpsimd.dma_start(w2m[:], w2_d[q * MT2 + m, :, :])
                    if q == 0 and m < 2 and anchor_act is not None:
                        # keep w2 prefetches out of the prologue DMA queues;
                        # they are only needed once phase 1 is well underway
                        add_dep_helper(w2dma.ins, anchor_act, sync=False,
                                       reason="delay w2 prefetch past early phase-1")
                    # on the very last output tile, finish with the smallest
                    # chunk: its evacuate+store is the kernel's tail
                    mchunks = chunks
                    if q == QF - 1 and m == MT2 - 1:
                        mchunks = sorted(chunks, key=lambda c: -c[1])
                    for ci, (cs, cw) in enumerate(mchunks):
                        ps = psp.tile([P, cw], f32, tag="ps", name=f"ps{ci}")
                        for kq in range(KQ2):
                            nc.tensor.matmul(
                                ps[:],
                                w2m[:, kq * P:(kq + 1) * P],
                                hT_sb[:, kq * cap + cs:kq * cap + cs + cw],
                                start=(kq == 0),
                                stop=(kq == KQ2 - 1),
                            )
                        ysl = y_sb[:, m * cap + cs:m * cap + cs + cw]
                        if q == 0:
                            nc.scalar.activation(ysl, ps[:], ident,
                                                 bias=b2_sb[:, m:m + 1])
                        elif q < QF - 1:
                            nc.vector.tensor_add(ysl, ps[:], ysl)
                        else:
                            ot = op.tile([P, cw], f32, tag="o", name=f"o{ci}")
                            nc.vector.tensor_add(ot[:], ps[:], ysl)
                            nc.sync.dma_start(y_d[m, :, cs:cs + cw], ot[:])

    nc.compile()
    return nc


def kernel(x, Wg, bg, W1, b1, W2, b2):
    global LAST_RESULT
    _ensure_axon_hooks()
    from concourse.bass_utils import run_bass_kernel_spmd

    x = np.ascontiguousarray(np.asarray(x, dtype=np.float32))
    Wg = np.asarray(Wg, dtype=np.float32)
    bg = np.asarray(bg, dtype=np.float32)
    W1 = np.asarray(W1, dtype=np.float32)
    b1 = np.asarray(b1, dtype=np.float32)
    W2 = np.asarray(W2, dtype=np.float32)
    b2 = np.asarray(b2, dtype=np.float32)

    B, S, D = x.shape
    T = B * S
    xf = x.reshape(T, D)

    top_idx, top_w = _route(xf, Wg, bg)

    tok_idx = []
    tok_w = []
    for e in range(NUM_EXPERTS):
        sel = top_idx == e                       # [T, K]
        rows = np.nonzero(sel.any(axis=1))[0]
        tok_idx.append(rows)
        tok_w.append((top_w * sel).sum(axis=1)[rows].astype(np.float32))

    maxc = max(len(r) for r in tok_idx)
    cap = max(256, -(-maxc // 16) * 16)  # 64B-aligned rows, minimal padding

    import os as _os
    p2_bf16 = bool(_os.environ.get("MOE_P2_BF16"))
    nc = _build_device_program(cap, p2_bf16)

    in_maps = []
    for e in range(NUM_EXPERTS):
        idx_pad = np.zeros(cap, dtype=np.int64)
        idx_pad[:len(tok_idx[e])] = tok_idx[e]
        xg = np.ascontiguousarray(xf[idx_pad].T).reshape(KT1, P, cap)
        w1t = np.ascontiguousarray(
            W1[e].reshape(KT1, P, MT1, P).transpose(2, 1, 0, 3)
        ).reshape(MT1, P, KT1 * P)
        w2t = np.ascontiguousarray(
            W2[e].reshape(QF, KT2 // QF, P, MT2, P).transpose(0, 3, 2, 1, 4)
        ).reshape(QF * MT2, P, (KT2 // QF) * P)
        if p2_bf16:
            import ml_dtypes
            w2t = w2t.astype(ml_dtypes.bfloat16)
        in_maps.append({
            "xg": xg,
            "w1t": w1t,
            "w2t": w2t,
            "b1m": np.ascontiguousarray(b1[e].reshape(MT1, P).T),
            "b2m": np.ascontiguousarray(b2[e].reshape(MT2, P).T),
        })

    import os
    trace_cores = None
    if os.environ.get("MOE_TRACE_ALL"):
        trace_cores = list(range(NUM_EXPERTS))
    res = run_bass_kernel_spmd(nc, in_maps, core_ids=list(range(NUM_EXPERTS)),
                               trace_cores=trace_cores)
    LAST_RESULT = res

    out = np.zeros((T, D), dtype=np.float32)
    for e in range(NUM_EXPERTS):
        n_e = len(tok_idx[e])
        if n_e == 0:
            continue
        yT = res.results[e]["yT"].reshape(D, cap)
        out[tok_idx[e]] += tok_w[e][:, None] * yT[:, :n_e].T
    return out.reshape(B, S, D)



# revision 7
# speedup vs baseline: 1.2974x; 1.2974x over previous
"""MoE layer (8 experts, top-2) on 8 Trainium2 NeuronCores, expert-parallel.

Strategy:
  - Host computes the (tiny) gating linear + softmax + top-2 routing,
    mirroring the reference ops so expert selection matches exactly.
  - Tokens are dispatched to their experts on the host (the all-to-all),
    one expert per NeuronCore. Each core runs the 1024->4096->1024 gelu
    FFN for its expert over its routed tokens (padded to a common
    capacity), with all matmuls in float32r at full PE rate.
  - Host applies combine weights and scatter-adds back to token order.

Device layout: activations are kept transposed ([feature, token]) so both
matmuls consume the natural [K, M] weight layout and the phase-1 output
feeds phase-2 without any on-chip transpose. The 4096-wide hidden dim is
processed in quarters: phase 1 produces a quarter of the hidden
activations for ALL tokens (SBUF-resident), phase 2 immediately folds it
into an SBUF-resident partial sum of y. Expert weights therefore stream
from HBM exactly once, which keeps the kernel tensor-engine-bound
(streaming them per token-group was HBM-bound under 8-core contention).
"""

import numpy as np

N_EMBED = 1024
FFN_HIDDEN = 4096
NUM_EXPERTS = 8
TOP_K = 2
P = 128
KT1 = N_EMBED // P       # 8  k-tiles, phase 1
MT1 = FFN_HIDDEN // P    # 32 m-tiles, phase 1
KT2 = FFN_HIDDEN // P    # 32 k-tiles, phase 2
MT2 = N_EMBED // P       # 8  m-tiles, phase 2
QF = 4                   # FFN hidden dim is processed in QF f-quarters so the
                         # resident hT slab stays small and weights stream once

LAST_RESULT = None       # BassKernelResults of the most recent run (debug/profiling)


def _ensure_axon_hooks():
    """Make `antenv.axon_hooks` importable so BASS_TRACE=1 degrades
    gracefully instead of crashing when the image lacks the module."""
    try:
        import antenv.axon_hooks  # noqa: F401
        return
    except ImportError:
        pass
    import sys
    import types

    m = types.ModuleType("antenv.axon_hooks")
    m._hook = None
    m.set_axon_ntff_profile_hook = lambda h: setattr(m, "_hook", h)
    m.get_axon_ntff_profile_hook = lambda: m._hook
    sys.modules["antenv.axon_hooks"] = m
    try:
        from trn_agent_boot.trn_boot import _ntff_profile_via_ctypes

        m.set_axon_ntff_profile_hook(_ntff_profile_via_ctypes("/opt/axon/libaxon_pjrt.so"))
    except Exception:
        pass


def _route(x2d, Wg, bg):
    """Top-2 gating. Mirrors the reference (jax softmax + lax.top_k) so the
    selected experts match it exactly; numpy fallback is numerically
    equivalent up to fp32 rounding."""
    try:
        import jax
        import jax.numpy as jnp

        gate = jax.nn.softmax(jnp.asarray(x2d) @ jnp.asarray(Wg) + jnp.asarray(bg), axis=-1)
        scores, idx = jax.lax.top_k(gate, TOP_K)
        scores = np.asarray(scores, dtype=np.float32)
        idx = np.asarray(idx)
    except Exception:
        logits = x2d @ Wg + bg
        m = logits.max(-1, keepdims=True)
        e = np.exp(logits - m)
        p = e / e.sum(-1, keepdims=True)
        order = np.argsort(-p, axis=-1, kind="stable")
        idx = order[:, :TOP_K]
        scores = np.take_along_axis(p, idx, axis=-1)
    scores = scores / (scores.sum(-1, keepdims=True) + 1e-8)
    return idx.astype(np.int64), scores.astype(np.float32)


def _chunks(width):
    """Split the token capacity (>=256) into matmul free-dim chunks, each
    in [256, 512] so every fp32r matmul runs at full PE rate. Smallest
    chunk first: it gates the very first matmul of the kernel."""
    out, rem = [], width
    while rem > 0:
        if rem <= 512:
            c = rem
        elif rem < 768:
            c = rem - 256
        else:
            c = 512
        out.append(c)
        rem -= c
    out.sort()
    res, off = [], 0
    for c in out:
        res.append((off, c))
        off += c
    return res




def _build_device_program(cap, use_bf16=True):
    import concourse.tile as tile
    from concourse import bacc, mybir
    from concourse.tile_rust import add_dep_helper

    f32 = mybir.dt.float32
    f32r = mybir.dt.float32r
    # bf16 everywhere: same 1 col/cycle PE rate as fp32r, but half the HBM
    # traffic (the fp32 kernel ran at ~90% HBM utilization under 8-core
    # contention) and fast-weight-load LDWEIGHTS (~53ns vs ~191ns for the
    # 4-byte path), which was partially exposed per matmul in fp32r.
    mmdt = mybir.dt.bfloat16 if use_bf16 else f32r
    p2dt = mmdt
    gelu = mybir.ActivationFunctionType.Gelu_apprx_tanh
    ident = mybir.ActivationFunctionType.Identity

    nc = bacc.Bacc("TRN2", target_bir_lowering=False, debug=False,
                   num_devices=NUM_EXPERTS)

    MQ1 = MT1 // QF  # phase-1 m-tiles (f-tiles) per quarter
    KQ2 = KT2 // QF  # phase-2 k-tiles (f-tiles) per quarter

    xg_d = nc.dram_tensor("xg", [KT1, P, cap], mmdt, kind="ExternalInput").ap()
    w1_d = nc.dram_tensor("w1t", [MT1, P, KT1 * P], mmdt, kind="ExternalInput").ap()
    # w2 is laid out quarter-sliced: tile (q*MT2 + m) holds the KQ2 k-slices
    # of f-quarter q for output tile m
    w2_d = nc.dram_tensor("w2t", [QF * MT2, P, KQ2 * P], p2dt,
                          kind="ExternalInput").ap()
    b1_d = nc.dram_tensor("b1m", [P, MT1], f32, kind="ExternalInput").ap()
    b2_d = nc.dram_tensor("b2m", [P, MT2], f32, kind="ExternalInput").ap()
    y_d = nc.dram_tensor("yT", [MT2, P, cap], mmdt, kind="ExternalOutput").ap()

    chunks = _chunks(cap)

    with tile.TileContext(nc) as tc:
        with (
            tc.tile_pool(name="const", bufs=1) as const,
            tc.tile_pool(name="xp", bufs=1) as xp,
            tc.tile_pool(name="hp", bufs=1) as hp,
            tc.tile_pool(name="yp", bufs=1) as yp,
            tc.tile_pool(name="w1p", bufs=4) as w1p,
            tc.tile_pool(name="w2p", bufs=4) as w2p,
            tc.tile_pool(name="psp", bufs=8, space="PSUM") as psp,
            tc.tile_pool(name="op", bufs=6) as op,
        ):
            # one tile per k-tile of x, split at the first chunk boundary;
            # all first-chunk slices are issued before anything else (DMA
            # issue on the sequencer costs ~650ns each) so the first
            # k-accumulation starts as soon as possible
            first_cw = chunks[0][1]
            xg_sbs = []
            for kt in range(KT1):
                xt = xp.tile([P, cap], mmdt, name=f"xg{kt}")
                nc.sync.dma_start(xt[:, :first_cw], xg_d[kt, :, :first_cw])
                xg_sbs.append(xt)
            b1_sb = const.tile([P, MT1], f32)
            nc.sync.dma_start(b1_sb[:], b1_d[:, :])
            b2_sb = const.tile([P, MT2], f32)
            nc.sync.dma_start(b2_sb[:], b2_d[:, :])
            for kt in range(KT1):
                nc.sync.dma_start(xg_sbs[kt][:, first_cw:], xg_d[kt, :, first_cw:])

            hT_sb = hp.tile([P, MQ1 * cap], p2dt)
            y_sb = yp.tile([P, MT2 * cap], f32)

            anchor_act = None
            for q in range(QF):
                # phase 1 (quarter q): hT = gelu(W1[:, fq].T @ xT + b1[fq])
                for mq in range(MQ1):
                    m = q * MQ1 + mq
                    w1m = w1p.tile([P, KT1 * P], mmdt, tag="w1")
                    nc.gpsimd.dma_start(w1m[:], w1_d[m, :, :])
                    for ci, (cs, cw) in enumerate(chunks):
                        ps = psp.tile([P, cw], f32, tag="ps", name=f"ps{ci}")
                        for kt in range(KT1):
                            nc.tensor.matmul(
                                ps[:],
                                w1m[:, kt * P:(kt + 1) * P],
                                xg_sbs[kt][:, cs:cs + cw],
                                start=(kt == 0),
                                stop=(kt == KT1 - 1),
                            )
                        act = nc.scalar.activation(
                            hT_sb[:, mq * cap + cs:mq * cap + cs + cw],
                            ps[:],
                            gelu,
                            bias=b1_sb[:, m:m + 1],
                        )
                        if q == 0 and mq == 6 and ci == 0:
                            anchor_act = act.ins
                # phase 2 (quarter q): y (+)= W2[fq].T @ hT  [+ b2 on q=0]
                for m in range(MT2):
                    w2m = w2p.tile([P, KQ2 * P], p2dt, tag="w2")
                    w2dma = nc.gpsimd.dma_start(w2m[:], w2_d[q * MT2 + m, :, :])
                    if q == 0 and m < 2 and anchor_act is not None:
                        # keep w2 prefetches out of the prologue DMA queues;
                        # they are only needed once phase 1 is well underway
                        add_dep_helper(w2dma.ins, anchor_act, sync=False,
                                       reason="delay w2 prefetch past early phase-1")
                    # on the very last output tile, finish with the smallest
                    # chunk: its evacuate+store is the kernel's tail
                    mchunks = chunks
                    if q == QF - 1 and m == MT2 - 1:
                        mchunks = sorted(chunks, key=lambda c: -c[1])
                    for ci, (cs, cw) in enumerate(mchunks):
                        ps = psp.tile([P, cw], f32, tag="ps", name=f"ps{ci}")
                        for kq in range(KQ2):
                            nc.tensor.matmul(
                                ps[:],
                                w2m[:, kq * P:(kq + 1) * P],
                                hT_sb[:, kq * cap + cs:kq * cap + cs + cw],
                                start=(kq == 0),
                                stop=(kq == KQ2 - 1),
                            )
                        ysl = y_sb[:, m * cap + cs:m * cap + cs + cw]
                        if q == 0:
                            nc.scalar.activation(ysl, ps[:], ident,
                                                 bias=b2_sb[:, m:m + 1])
                        elif q < QF - 1:
                            nc.vector.tensor_add(ysl, ps[:], ysl)
                        else:
                            ot = op.tile([P, cw], mmdt, tag="o", name=f"o{ci}")
                            nc.vector.tensor_add(ot[:], ps[:], ysl)
                            nc.sync.dma_start(y_d[m, :, cs:cs + cw], ot[:])

    nc.compile()
    return nc


def kernel(x, Wg, bg, W1, b1, W2, b2):
    global LAST_RESULT
    _ensure_axon_hooks()
    from concourse.bass_utils import run_bass_kernel_spmd

    x = np.ascontiguousarray(np.asarray(x, dtype=np.float32))
    Wg = np.asarray(Wg, dtype=np.float32)
    bg = np.asarray(bg, dtype=np.float32)
    W1 = np.asarray(W1, dtype=np.float32)
    b1 = np.asarray(b1, dtype=np.float32)
    W2 = np.asarray(W2, dtype=np.float32)
    b2 = np.asarray(b2, dtype=np.float32)

    B, S, D = x.shape
    T = B * S
    xf = x.reshape(T, D)

    top_idx, top_w = _route(xf, Wg, bg)

    tok_idx = []
    tok_w = []
    for e in range(NUM_EXPERTS):
        sel = top_idx == e                       # [T, K]
        rows = np.nonzero(sel.any(axis=1))[0]
        tok_idx.append(rows)
        tok_w.append((top_w * sel).sum(axis=1)[rows].astype(np.float32))

    maxc = max(len(r) for r in tok_idx)
    cap = max(256, -(-maxc // 16) * 16)  # 64B-aligned rows, minimal padding

    import os as _os
    use_bf16 = not bool(_os.environ.get("MOE_FP32"))
    nc = _build_device_program(cap, use_bf16)

    import ml_dtypes
    mmdt_np = ml_dtypes.bfloat16 if use_bf16 else np.float32

    in_maps = []
    for e in range(NUM_EXPERTS):
        idx_pad = np.zeros(cap, dtype=np.int64)
        idx_pad[:len(tok_idx[e])] = tok_idx[e]
        xg = np.ascontiguousarray(xf[idx_pad].T).reshape(KT1, P, cap)
        w1t = np.ascontiguousarray(
            W1[e].reshape(KT1, P, MT1, P).transpose(2, 1, 0, 3)
        ).reshape(MT1, P, KT1 * P)
        w2t = np.ascontiguousarray(
            W2[e].reshape(QF, KT2 // QF, P, MT2, P).transpose(0, 3, 2, 1, 4)
        ).reshape(QF * MT2, P, (KT2 // QF) * P)
        in_maps.append({
            "xg": xg.astype(mmdt_np),
            "w1t": w1t.astype(mmdt_np),
            "w2t": w2t.astype(mmdt_np),
            "b1m": np.ascontiguousarray(b1[e].reshape(MT1, P).T),
            "b2m": np.ascontiguousarray(b2[e].reshape(MT2, P).T),
        })

    import os
    trace_cores = None
    if os.environ.get("MOE_TRACE_ALL"):
        trace_cores = list(range(NUM_EXPERTS))
    res = run_bass_kernel_spmd(nc, in_maps, core_ids=list(range(NUM_EXPERTS)),
                               trace_cores=trace_cores)
    LAST_RESULT = res

    out = np.zeros((T, D), dtype=np.float32)
    for e in range(NUM_EXPERTS):
        n_e = len(tok_idx[e])
        if n_e == 0:
            continue
        yT = np.asarray(res.results[e]["yT"], dtype=np.float32).reshape(D, cap)
        out[tok_idx[e]] += tok_w[e][:, None] * yT[:, :n_e].T
    return out.reshape(B, S, D)

